# revision 29
# baseline (speedup 1.0000x reference)
"""Trainium2 Bass kernel for nn_Colorcal_TwoDatasets (per-sample affine color
calibration with per-(cam,id,dataset) gathered scale/bias).

Contract: kernel(**inputs) takes the FULL unsharded inputs (see shapes below),
shards the batch across 8 NeuronCores (2 samples per core, pure data parallel),
runs a Bass/Tile kernel per core, and gathers the full [16,3,1024,1024] output.

Device kernel per core:
  - the (cam,id,dataset) gather runs on-device on 12 partitions (one per
    gathered scale/bias value): masked one-hot compares against an iota over
    the concatenated tables, one tensor_mul + tensor_reduce, then a tiny
    SBUF->SBUF transpose DMA + gpsimd partition_broadcast produce [128,12]
    per-partition scale/bias operands
  - the 24 MiB image shard is streamed plane-by-plane through SBUF (one 4 MiB
    HWDGE DMA per plane, triple-buffered) with one fused multiply-add per
    plane, alternating DVE tensor_scalar / ACT activation(Identity)
"""

import numpy as np

import concourse.bacc as bacc
import concourse.mybir as mybir
import concourse.tile as tile
from concourse import bass_utils

N_CORES = 8
B, C, H, W = 16, 3, 1024, 1024
BPC = B // N_CORES  # samples per core
NC1, NI1, NC2, NI2 = 40, 256, 80, 512
SEG = NC1 + NI1 + NC2 + NI2  # 888: [cam1 | ident1 | cam2 | ident2]
PF = H * W // 128  # 8192 free elements per plane per partition
TILE_F = 8192  # free-dim tile size: full plane per DMA (4 MiB), best HBM BW
F32 = mybir.dt.float32

_CACHE = {}

_SEGS = (
    # (start, end, idx_col) over the concatenated [cam1|ident1|cam2|ident2] axis;
    # idx_col: 0=cam, 1=id; mask: 0 -> dataset==0 segment, 1 -> dataset==1
    (0, NC1, 0, 0),
    (NC1, NC1 + NI1, 1, 0),
    (NC1 + NI1, NC1 + NI1 + NC2, 0, 1),
    (NC1 + NI1 + NC2, SEG, 1, 1),
)


def _gather12(nc, cpool, spool, aux, wb_t, NR):
    """Gather on NR=12 partitions (one row per output value), then broadcast.
    Row r = off*6 + i*3 + c carries sample i(r)'s indices and the (w|b, c)
    table slice; one mul+reduce computes all 12 dot products at once.
    aux columns: [0:4) idx(cam,id,dt,-), [4:4+SEG) iota, [4+SEG:4+2*SEG) table."""
    mult = mybir.AluOpType.mult
    add = mybir.AluOpType.add
    iseq = mybir.AluOpType.is_equal
    aux_t = cpool.tile([NR, 4 + 2 * SEG], F32)
    nc.sync.dma_start(out=aux_t[:], in_=aux[:])
    idx_t = aux_t[:, 0:4]
    iota_t = aux_t[:, 4 : 4 + SEG]
    wbtab_t = aux_t[:, 4 + SEG : 4 + 2 * SEG]

    m_t = cpool.tile([NR, 2], F32)
    nc.vector.tensor_scalar(out=m_t[:, 0:1], in0=idx_t[:, 2:3],
                            scalar1=0.0, scalar2=None, op0=iseq)
    nc.vector.tensor_scalar(out=m_t[:, 1:2], in0=idx_t[:, 2:3],
                            scalar1=1.0, scalar2=None, op0=iseq)
    oh = spool.tile([NR, SEG], F32, tag="oh")
    for a, b, col, mcol in _SEGS:
        nc.vector.tensor_scalar(
            out=oh[:, a:b], in0=iota_t[:, a:b],
            scalar1=idx_t[:, col : col + 1],
            scalar2=m_t[:, mcol : mcol + 1],
            op0=iseq, op1=mult,
        )
    prod = spool.tile([NR, SEG], F32, tag="prod")
    nc.vector.tensor_mul(out=prod[:], in0=oh[:], in1=wbtab_t[:])
    wbp = cpool.tile([NR, 1], F32)
    nc.vector.tensor_reduce(out=wbp[:], in_=prod[:],
                            axis=mybir.AxisListType.X, op=add)
    # transpose [NR,1] -> [1,NR] (tiny SBUF->SBUF DMA), then broadcast to all
    # 128 partitions for use as per-partition scale/bias operands
    wbrow = cpool.tile([1, NR], F32)
    nc.sync.dma_start(out=wbrow[:], in_=wbp[:])
    nc.gpsimd.partition_broadcast(wb_t[:], wbrow[:])


def _gather128(nc, cpool, spool, idx, iotas, wtab, btab, wb_t):
    """Original variant: tables replicated across 128 partitions."""
    mult = mybir.AluOpType.mult
    add = mybir.AluOpType.add
    iseq = mybir.AluOpType.is_equal
    idx_t = cpool.tile([128, 3 * BPC], F32)
    nc.sync.dma_start(out=idx_t[:], in_=idx[:])
    iota_t = cpool.tile([128, SEG], F32)
    nc.sync.dma_start(out=iota_t[:], in_=iotas[:])
    wtab_t = cpool.tile([128, C * SEG], F32)
    nc.sync.dma_start(out=wtab_t[:], in_=wtab[:])
    btab_t = cpool.tile([128, C * SEG], F32)
    nc.sync.dma_start(out=btab_t[:], in_=btab[:])
    m_t = cpool.tile([128, 2 * BPC], F32)
    for i in range(BPC):
        dc = 3 * i + 2
        nc.vector.tensor_scalar(
            out=m_t[:, 2 * i : 2 * i + 1], in0=idx_t[:, dc : dc + 1],
            scalar1=0.0, scalar2=None, op0=iseq,
        )
        nc.vector.tensor_scalar(
            out=m_t[:, 2 * i + 1 : 2 * i + 2], in0=idx_t[:, dc : dc + 1],
            scalar1=1.0, scalar2=None, op0=iseq,
        )
        oh = spool.tile([128, SEG], F32, tag="oh")
        for a, b, col, mcol in _SEGS:
            nc.vector.tensor_scalar(
                out=oh[:, a:b], in0=iota_t[:, a:b],
                scalar1=idx_t[:, 3 * i + col : 3 * i + col + 1],
                scalar2=m_t[:, 2 * i + mcol : 2 * i + mcol + 1],
                op0=iseq, op1=mult,
            )
        for c in range(C):
            for tab_t, off in ((wtab_t, 0), (btab_t, BPC * C)):
                # NOTE: tensor_tensor_reduce wedges this HW/ucode
                # (NRT_EXEC_UNIT_UNRECOVERABLE); use mul + reduce.
                prod = spool.tile([128, SEG], F32, tag="prod")
                nc.vector.tensor_mul(
                    out=prod[:], in0=oh[:],
                    in1=tab_t[:, c * SEG : (c + 1) * SEG],
                )
                nc.vector.tensor_reduce(
                    out=wb_t[:, off + i * C + c : off + i * C + c + 1],
                    in_=prod[:], axis=mybir.AxisListType.X, op=add,
                )


def _build(reps: int = 1, tile_f: int = TILE_F, bufs: int = 4, mix: str = "alt",
           gmode: str = "12", store_eng: str = "sp"):
    """Build the per-core program. reps>1 repeats the streaming stage (used
    only for timing measurements — differencing two rep counts cancels the
    dispatch overhead and one-time costs). mix: 'alt' alternates DVE/ACT for
    the affine, 'dve' uses DVE only, 'act' ACT only. gmode: '12' computes the
    gather on 12 partitions + broadcasts (tiny aux inputs); '128' replicates
    the tables across all partitions."""
    key = ("nc", reps, tile_f, bufs, mix, gmode, store_eng)
    if key in _CACHE:
        return _CACHE[key]
    nc = bacc.Bacc("TRN2", target_bir_lowering=False, debug=False, num_devices=N_CORES)
    NR = 2 * BPC * C  # 12 gathered values: r = off*BPC*C + i*C + c (off: 0=w 1=b)
    img = nc.dram_tensor("img", [BPC, C, H, W], F32, kind="ExternalInput").ap()
    if gmode == "12":
        aux = nc.dram_tensor("aux", [NR, 4 + 2 * SEG], F32, kind="ExternalInput").ap()
    else:
        idx = nc.dram_tensor("idx", [128, 3 * BPC], F32, kind="ExternalInput").ap()
        iotas = nc.dram_tensor("iotas", [128, SEG], F32, kind="ExternalInput").ap()
        wtab = nc.dram_tensor("wtab", [128, C * SEG], F32, kind="ExternalInput").ap()
        btab = nc.dram_tensor("btab", [128, C * SEG], F32, kind="ExternalInput").ap()
    out = nc.dram_tensor("out", [BPC, C, H, W], F32, kind="ExternalOutput").ap()

    mult = mybir.AluOpType.mult
    add = mybir.AluOpType.add
    iseq = mybir.AluOpType.is_equal

    with tile.TileContext(nc) as tc:
        with (
            tc.tile_pool(name="const", bufs=1) as cpool,
            tc.tile_pool(name="scratch", bufs=2) as spool,
            tc.tile_pool(name="io", bufs=bufs) as iopool,
        ):
            # gathered affine params: w at col i*C+c, b at col BPC*C + i*C+c
            wb_t = cpool.tile([128, NR], F32)
            if gmode == "12":
                _gather12(nc, cpool, spool, aux, wb_t, NR)
            else:
                _gather128(nc, cpool, spool, idx, iotas, wtab, btab, wb_t)

            nplanes = BPC * C

            def plane_sizes(pidx):
                if not isinstance(tile_f, str):
                    return [tile_f] * (PF // tile_f)
                # ramped schedules: smaller tiles at the very start (fast
                # pipeline fill) and very end (fast drain), full planes between
                first, last = {
                    "ramp": ([2048, 2048, 4096], [4096, 2048, 2048]),
                    "ramp2": ([2048, 6144], [6144, 2048]),
                    "ramp3": ([4096, 4096], [4096, 4096]),
                }[tile_f]
                if pidx == 0:
                    return first
                if pidx == nplanes - 1:
                    return last
                return [PF]

            store = nc.scalar if store_eng == "act" else nc.sync

            def affine(ap, w_ap, b_ap, k):
                use_dve = mix == "dve" or (mix == "alt" and k % 2 == 0)
                if use_dve:
                    nc.vector.tensor_scalar(
                        out=ap, in0=ap,
                        scalar1=w_ap, scalar2=b_ap, op0=mult, op1=add,
                    )
                else:
                    nc.scalar.activation(
                        out=ap, in_=ap,
                        func=mybir.ActivationFunctionType.Identity,
                        bias=b_ap, scale=w_ap,
                    )

            def w_b(plane):
                i, c = divmod(plane, C)
                return (
                    wb_t[:, i * C + c : i * C + c + 1],
                    wb_t[:, BPC * C + i * C + c : BPC * C + i * C + c + 1],
                )

            max_tf = PF if isinstance(tile_f, str) else tile_f
            # k starts at 1: the first plane's affine lands on ACT, which is
            # idle while the gather chain occupies DVE
            k = 1
            for _rep in range(reps):
              for plane in range(nplanes):
                i, c = divmod(plane, C)
                src = img[i, c].rearrange("(p r) w -> p (r w)", p=128)
                dst = out[i, c].rearrange("(p r) w -> p (r w)", p=128)
                w_ap, b_ap = w_b(plane)
                pos = 0
                for sz in plane_sizes(plane):
                    tl = iopool.tile([128, max_tf], F32, tag="io")
                    nc.sync.dma_start(
                        out=tl[:, :sz], in_=src[:, pos : pos + sz]
                    )
                    affine(tl[:, :sz], w_ap, b_ap, k)
                    store.dma_start(
                        out=dst[:, pos : pos + sz], in_=tl[:, :sz]
                    )
                    pos += sz
                    k += 1

    nc.compile()
    _CACHE[key] = nc
    return nc


def make_in_maps(image, camindex, idindex, dataset_type,
                 wcam1, bcam1, wident1, bident1,
                 wcam2, bcam2, wident2, bident2, gmode: str = "12"):
    """Host-side sharding + layout: batch-shard the image/indices, replicate
    the tiny tables (pure data movement; all gather math runs on device)."""
    image = np.ascontiguousarray(np.asarray(image, dtype=np.float32))
    cam = np.asarray(camindex).astype(np.float32)
    idi = np.asarray(idindex).astype(np.float32)
    dts = np.asarray(dataset_type).astype(np.float32)

    iot = np.concatenate(
        [np.arange(NC1), np.arange(NI1), np.arange(NC2), np.arange(NI2)]
    ).astype(np.float32)
    wrow = np.concatenate(
        [np.asarray(t, dtype=np.float32) for t in (wcam1, wident1, wcam2, wident2)],
        axis=0,
    )  # [SEG, 3]
    brow = np.concatenate(
        [np.asarray(t, dtype=np.float32) for t in (bcam1, bident1, bcam2, bident2)],
        axis=0,
    )

    NR = 2 * BPC * C
    in_maps = []
    if gmode == "12":
        # one aux tensor per core: [0:4) idx, [4:4+SEG) iota, [4+SEG:) table
        # row r = off*BPC*C + i*C + c: table (w if off==0 else b), channel c
        aux0 = np.zeros((NR, 4 + 2 * SEG), np.float32)
        aux0[:, 4 : 4 + SEG] = iot
        for r in range(NR):
            off, rem = divmod(r, BPC * C)
            i, c = divmod(rem, C)
            aux0[r, 4 + SEG :] = (wrow if off == 0 else brow)[:, c]
        for k in range(N_CORES):
            s = slice(BPC * k, BPC * (k + 1))
            aux = aux0.copy()
            for r in range(NR):
                off, rem = divmod(r, BPC * C)
                i, c = divmod(rem, C)
                gi = BPC * k + i
                aux[r, 0] = cam[gi]
                aux[r, 1] = idi[gi]
                aux[r, 2] = dts[gi]
            in_maps.append({"img": image[s], "aux": aux})
    else:
        iotas = np.ascontiguousarray(np.broadcast_to(iot, (128, SEG)))
        wtab = np.ascontiguousarray(
            np.broadcast_to(wrow.T.reshape(-1), (128, C * SEG))
        )
        btab = np.ascontiguousarray(
            np.broadcast_to(brow.T.reshape(-1), (128, C * SEG))
        )
        for k in range(N_CORES):
            s = slice(BPC * k, BPC * (k + 1))
            row = np.stack([cam[s], idi[s], dts[s]], axis=1).reshape(-1)
            idx = np.ascontiguousarray(np.broadcast_to(row, (128, 3 * BPC)))
            in_maps.append(
                {"img": image[s], "idx": idx, "iotas": iotas,
                 "wtab": wtab, "btab": btab}
            )
    return in_maps


def kernel(image, camindex, idindex, dataset_type,
           wcam1, bcam1, wident1, bident1,
           wcam2, bcam2, wident2, bident2) -> np.ndarray:
    nc = _build()
    in_maps = make_in_maps(
        image, camindex, idindex, dataset_type,
        wcam1, bcam1, wident1, bident1, wcam2, bcam2, wident2, bident2,
    )
    res = bass_utils.run_bass_kernel_spmd(nc, in_maps, list(range(N_CORES)))
    return np.concatenate(
        [res.results[k]["out"] for k in range(N_CORES)], axis=0
    )


# revision 31
# speedup vs baseline: 1.1022x; 1.1022x over previous
"""Trainium2 Bass kernel for nn_Colorcal_TwoDatasets (per-sample affine color
calibration with per-(cam,id,dataset) gathered scale/bias).

Contract: kernel(**inputs) takes the FULL unsharded inputs (see shapes below),
shards the batch across 8 NeuronCores (2 samples per core, pure data parallel),
runs a Bass/Tile kernel per core, and gathers the full [16,3,1024,1024] output.

Device kernel per core:
  - the (cam,id,dataset) gather runs on-device on 12 partitions (one per
    gathered scale/bias value): masked one-hot compares against an iota over
    the concatenated tables, one tensor_mul + tensor_reduce, then a tiny
    SBUF->SBUF transpose DMA + gpsimd partition_broadcast produce [128,12]
    per-partition scale/bias operands
  - the 24 MiB image shard is streamed plane-by-plane through SBUF (one 4 MiB
    HWDGE DMA per plane, triple-buffered) with one fused multiply-add per
    plane, alternating DVE tensor_scalar / ACT activation(Identity)
"""

import numpy as np

import concourse.bacc as bacc
import concourse.mybir as mybir
import concourse.tile as tile
from concourse import bass_utils

N_CORES = 8
B, C, H, W = 16, 3, 1024, 1024
BPC = B // N_CORES  # samples per core
NC1, NI1, NC2, NI2 = 40, 256, 80, 512
SEG = NC1 + NI1 + NC2 + NI2  # 888: [cam1 | ident1 | cam2 | ident2]
PF = H * W // 128  # 8192 free elements per plane per partition
TILE_F = 8192  # free-dim tile size: full plane per DMA (4 MiB), best HBM BW
F32 = mybir.dt.float32

_CACHE = {}

_SEGS = (
    # (start, end, idx_col) over the concatenated [cam1|ident1|cam2|ident2] axis;
    # idx_col: 0=cam, 1=id; mask: 0 -> dataset==0 segment, 1 -> dataset==1
    (0, NC1, 0, 0),
    (NC1, NC1 + NI1, 1, 0),
    (NC1 + NI1, NC1 + NI1 + NC2, 0, 1),
    (NC1 + NI1 + NC2, SEG, 1, 1),
)


def _gather12(nc, cpool, spool, aux, wb_t, NR):
    """Gather on NR=12 partitions (one row per output value), then broadcast.
    Row r = off*6 + i*3 + c carries sample i(r)'s indices and the (w|b, c)
    table slice; one mul+reduce computes all 12 dot products at once.
    aux columns: [0:4) idx(cam,id,dt,-), [4:4+SEG) iota, [4+SEG:4+2*SEG) table."""
    mult = mybir.AluOpType.mult
    add = mybir.AluOpType.add
    iseq = mybir.AluOpType.is_equal
    aux_t = cpool.tile([NR, 4 + 2 * SEG], F32)
    nc.sync.dma_start(out=aux_t[:], in_=aux[:])
    idx_t = aux_t[:, 0:4]
    iota_t = aux_t[:, 4 : 4 + SEG]
    wbtab_t = aux_t[:, 4 + SEG : 4 + 2 * SEG]

    m_t = cpool.tile([NR, 2], F32)
    nc.vector.tensor_scalar(out=m_t[:, 0:1], in0=idx_t[:, 2:3],
                            scalar1=0.0, scalar2=None, op0=iseq)
    nc.vector.tensor_scalar(out=m_t[:, 1:2], in0=idx_t[:, 2:3],
                            scalar1=1.0, scalar2=None, op0=iseq)
    oh = spool.tile([NR, SEG], F32, tag="oh")
    for a, b, col, mcol in _SEGS:
        nc.vector.tensor_scalar(
            out=oh[:, a:b], in0=iota_t[:, a:b],
            scalar1=idx_t[:, col : col + 1],
            scalar2=m_t[:, mcol : mcol + 1],
            op0=iseq, op1=mult,
        )
    prod = spool.tile([NR, SEG], F32, tag="prod")
    nc.vector.tensor_mul(out=prod[:], in0=oh[:], in1=wbtab_t[:])
    wbp = cpool.tile([NR, 1], F32)
    nc.vector.tensor_reduce(out=wbp[:], in_=prod[:],
                            axis=mybir.AxisListType.X, op=add)
    # transpose [NR,1] -> [1,NR] (tiny SBUF->SBUF DMA), then broadcast to all
    # 128 partitions for use as per-partition scale/bias operands
    wbrow = cpool.tile([1, NR], F32)
    nc.sync.dma_start(out=wbrow[:], in_=wbp[:])
    nc.gpsimd.partition_broadcast(wb_t[:], wbrow[:])


def _gather128(nc, cpool, spool, idx, iotas, wtab, btab, wb_t):
    """Original variant: tables replicated across 128 partitions."""
    mult = mybir.AluOpType.mult
    add = mybir.AluOpType.add
    iseq = mybir.AluOpType.is_equal
    idx_t = cpool.tile([128, 3 * BPC], F32)
    nc.sync.dma_start(out=idx_t[:], in_=idx[:])
    iota_t = cpool.tile([128, SEG], F32)
    nc.sync.dma_start(out=iota_t[:], in_=iotas[:])
    wtab_t = cpool.tile([128, C * SEG], F32)
    nc.sync.dma_start(out=wtab_t[:], in_=wtab[:])
    btab_t = cpool.tile([128, C * SEG], F32)
    nc.sync.dma_start(out=btab_t[:], in_=btab[:])
    m_t = cpool.tile([128, 2 * BPC], F32)
    for i in range(BPC):
        dc = 3 * i + 2
        nc.vector.tensor_scalar(
            out=m_t[:, 2 * i : 2 * i + 1], in0=idx_t[:, dc : dc + 1],
            scalar1=0.0, scalar2=None, op0=iseq,
        )
        nc.vector.tensor_scalar(
            out=m_t[:, 2 * i + 1 : 2 * i + 2], in0=idx_t[:, dc : dc + 1],
            scalar1=1.0, scalar2=None, op0=iseq,
        )
        oh = spool.tile([128, SEG], F32, tag="oh")
        for a, b, col, mcol in _SEGS:
            nc.vector.tensor_scalar(
                out=oh[:, a:b], in0=iota_t[:, a:b],
                scalar1=idx_t[:, 3 * i + col : 3 * i + col + 1],
                scalar2=m_t[:, 2 * i + mcol : 2 * i + mcol + 1],
                op0=iseq, op1=mult,
            )
        for c in range(C):
            for tab_t, off in ((wtab_t, 0), (btab_t, BPC * C)):
                # NOTE: tensor_tensor_reduce wedges this HW/ucode
                # (NRT_EXEC_UNIT_UNRECOVERABLE); use mul + reduce.
                prod = spool.tile([128, SEG], F32, tag="prod")
                nc.vector.tensor_mul(
                    out=prod[:], in0=oh[:],
                    in1=tab_t[:, c * SEG : (c + 1) * SEG],
                )
                nc.vector.tensor_reduce(
                    out=wb_t[:, off + i * C + c : off + i * C + c + 1],
                    in_=prod[:], axis=mybir.AxisListType.X, op=add,
                )


def _build(reps: int = 1, tile_f: int = TILE_F, bufs: int = 4, mix: str = "alt",
           gmode: str = "12", store_eng: str = "sp"):
    """Build the per-core program. reps>1 repeats the streaming stage (used
    only for timing measurements — differencing two rep counts cancels the
    dispatch overhead and one-time costs). mix: 'alt' alternates DVE/ACT for
    the affine, 'dve' uses DVE only, 'act' ACT only. gmode: '12' computes the
    gather on 12 partitions + broadcasts (tiny aux inputs); '128' replicates
    the tables across all partitions."""
    key = ("nc", reps, tile_f, bufs, mix, gmode, store_eng)
    if key in _CACHE:
        return _CACHE[key]
    nc = bacc.Bacc("TRN2", target_bir_lowering=False, debug=False, num_devices=N_CORES)
    NR = 2 * BPC * C  # 12 gathered values: r = off*BPC*C + i*C + c (off: 0=w 1=b)
    img = nc.dram_tensor("img", [BPC, C, H, W], F32, kind="ExternalInput").ap()
    if gmode == "12":
        aux = nc.dram_tensor("aux", [NR, 4 + 2 * SEG], F32, kind="ExternalInput").ap()
    else:
        idx = nc.dram_tensor("idx", [128, 3 * BPC], F32, kind="ExternalInput").ap()
        iotas = nc.dram_tensor("iotas", [128, SEG], F32, kind="ExternalInput").ap()
        wtab = nc.dram_tensor("wtab", [128, C * SEG], F32, kind="ExternalInput").ap()
        btab = nc.dram_tensor("btab", [128, C * SEG], F32, kind="ExternalInput").ap()
    out = nc.dram_tensor("out", [BPC, C, H, W], F32, kind="ExternalOutput").ap()

    mult = mybir.AluOpType.mult
    add = mybir.AluOpType.add
    iseq = mybir.AluOpType.is_equal

    with tile.TileContext(nc) as tc:
        with (
            tc.tile_pool(name="const", bufs=1) as cpool,
            tc.tile_pool(name="scratch", bufs=2) as spool,
            tc.tile_pool(name="io", bufs=bufs) as iopool,
        ):
            # gathered affine params: w at col i*C+c, b at col BPC*C + i*C+c
            wb_t = cpool.tile([128, NR], F32)
            if gmode == "12":
                _gather12(nc, cpool, spool, aux, wb_t, NR)
            else:
                _gather128(nc, cpool, spool, idx, iotas, wtab, btab, wb_t)

            nplanes = BPC * C

            def plane_sizes(pidx):
                if not isinstance(tile_f, str):
                    return [tile_f] * (PF // tile_f)
                # ramped schedules: smaller tiles at the very start (fast
                # pipeline fill) and very end (fast drain), full planes between
                first, last = {
                    "ramp": ([2048, 2048, 4096], [4096, 2048, 2048]),
                    "ramp2": ([2048, 6144], [6144, 2048]),
                    "ramp3": ([4096, 4096], [4096, 4096]),
                }[tile_f]
                if pidx == 0:
                    return first
                if pidx == nplanes - 1:
                    return last
                return [PF]

            store = nc.scalar if store_eng == "act" else nc.sync

            def affine(ap, w_ap, b_ap, k):
                use_dve = mix == "dve" or (mix == "alt" and k % 2 == 0)
                if use_dve:
                    nc.vector.tensor_scalar(
                        out=ap, in0=ap,
                        scalar1=w_ap, scalar2=b_ap, op0=mult, op1=add,
                    )
                else:
                    nc.scalar.activation(
                        out=ap, in_=ap,
                        func=mybir.ActivationFunctionType.Identity,
                        bias=b_ap, scale=w_ap,
                    )

            def w_b(plane):
                i, c = divmod(plane, C)
                return (
                    wb_t[:, i * C + c : i * C + c + 1],
                    wb_t[:, BPC * C + i * C + c : BPC * C + i * C + c + 1],
                )

            max_tf = PF if isinstance(tile_f, str) else tile_f
            # k starts at 1: the first plane's affine lands on ACT, which is
            # idle while the gather chain occupies DVE
            k = 1
            for _rep in range(reps):
              for plane in range(nplanes):
                i, c = divmod(plane, C)
                src = img[i, c].rearrange("(p r) w -> p (r w)", p=128)
                dst = out[i, c].rearrange("(p r) w -> p (r w)", p=128)
                w_ap, b_ap = w_b(plane)
                pos = 0
                for sz in plane_sizes(plane):
                    tl = iopool.tile([128, max_tf], F32, tag="io")
                    nc.sync.dma_start(
                        out=tl[:, :sz], in_=src[:, pos : pos + sz]
                    )
                    affine(tl[:, :sz], w_ap, b_ap, k)
                    store.dma_start(
                        out=dst[:, pos : pos + sz], in_=tl[:, :sz]
                    )
                    pos += sz
                    k += 1

    nc.compile()
    _CACHE[key] = nc
    return nc


def make_in_maps(image, camindex, idindex, dataset_type,
                 wcam1, bcam1, wident1, bident1,
                 wcam2, bcam2, wident2, bident2, gmode: str = "12"):
    """Host-side sharding + layout: batch-shard the image/indices, replicate
    the tiny tables (pure data movement; all gather math runs on device)."""
    image = np.ascontiguousarray(np.asarray(image, dtype=np.float32))
    cam = np.asarray(camindex).astype(np.float32)
    idi = np.asarray(idindex).astype(np.float32)
    dts = np.asarray(dataset_type).astype(np.float32)

    iot = np.concatenate(
        [np.arange(NC1), np.arange(NI1), np.arange(NC2), np.arange(NI2)]
    ).astype(np.float32)
    wrow = np.concatenate(
        [np.asarray(t, dtype=np.float32) for t in (wcam1, wident1, wcam2, wident2)],
        axis=0,
    )  # [SEG, 3]
    brow = np.concatenate(
        [np.asarray(t, dtype=np.float32) for t in (bcam1, bident1, bcam2, bident2)],
        axis=0,
    )

    NR = 2 * BPC * C
    in_maps = []
    if gmode == "12":
        # one aux tensor per core: [0:4) idx, [4:4+SEG) iota, [4+SEG:) table
        # row r = off*BPC*C + i*C + c: table (w if off==0 else b), channel c
        aux0 = np.zeros((NR, 4 + 2 * SEG), np.float32)
        aux0[:, 4 : 4 + SEG] = iot
        for r in range(NR):
            off, rem = divmod(r, BPC * C)
            i, c = divmod(rem, C)
            aux0[r, 4 + SEG :] = (wrow if off == 0 else brow)[:, c]
        for k in range(N_CORES):
            s = slice(BPC * k, BPC * (k + 1))
            aux = aux0.copy()
            for r in range(NR):
                off, rem = divmod(r, BPC * C)
                i, c = divmod(rem, C)
                gi = BPC * k + i
                aux[r, 0] = cam[gi]
                aux[r, 1] = idi[gi]
                aux[r, 2] = dts[gi]
            in_maps.append({"img": image[s], "aux": aux})
    else:
        iotas = np.ascontiguousarray(np.broadcast_to(iot, (128, SEG)))
        wtab = np.ascontiguousarray(
            np.broadcast_to(wrow.T.reshape(-1), (128, C * SEG))
        )
        btab = np.ascontiguousarray(
            np.broadcast_to(brow.T.reshape(-1), (128, C * SEG))
        )
        for k in range(N_CORES):
            s = slice(BPC * k, BPC * (k + 1))
            row = np.stack([cam[s], idi[s], dts[s]], axis=1).reshape(-1)
            idx = np.ascontiguousarray(np.broadcast_to(row, (128, 3 * BPC)))
            in_maps.append(
                {"img": image[s], "idx": idx, "iotas": iotas,
                 "wtab": wtab, "btab": btab}
            )
    return in_maps


def kernel(image, camindex, idindex, dataset_type,
           wcam1, bcam1, wident1, bident1,
           wcam2, bcam2, wident2, bident2) -> np.ndarray:
    nc = _build()
    in_maps = make_in_maps(
        image, camindex, idindex, dataset_type,
        wcam1, bcam1, wident1, bident1, wcam2, bcam2, wident2, bident2,
    )
    res = bass_utils.run_bass_kernel_spmd(nc, in_maps, list(range(N_CORES)))
    return np.concatenate(
        [res.results[k]["out"] for k in range(N_CORES)], axis=0
    )
